# revision 14
# baseline (speedup 1.0000x reference)
"""Trainium2 Bass kernel for CryptoAttentionLayer.

Computation (per batch element b, per token t):
    Q = x @ Wq + bq ; K = x @ Wk + bk ; V = x @ Wv + bv    (reshaped to 4 heads x 256)
    S[h,g]   = Q_h . K_g / 16                               (per-token 4x4 scores)
    W        = softmax_g(S)
    att_h    = sum_g W[h,g] * V_g
    out      = att @ Wo + bo
Sharding: data-parallel over B=8 across 8 NeuronCores; weights replicated.

v3: Q/K projections run in fp8-e4m3 DoubleRow mode (2 contraction rows per
cycle -> ~2x the bf16 matmul rate; exact e6m3 products, fp32 accumulate).
Wq/Wk are host-scaled by 64 so their +-1/32 entries leave fp8's subnormal
range; the score scale absorbs the 64^2. The Q/K bias rides as a 5th
DoubleRow matmul (ones/zero contraction chunks). V/O projections stay bf16
(their error hits the output linearly). Per-token attention runs on the DVE.
"""

import math

import numpy as np
import ml_dtypes

import concourse.bass as bass
import concourse.tile as tile
import concourse.mybir as mybir
from concourse import bacc
from concourse.bass_utils import run_bass_kernel_spmd
from concourse.masks import make_identity

B, N, D = 8, 4096, 1024
NUM_HEADS, HEAD_DIM = 4, 256
P = 128
NT = N // P          # 32 token tiles per core
KC = D // P          # 8 contraction chunks of 128
F32 = mybir.dt.float32
BF16 = mybir.dt.bfloat16
FP8 = mybir.dt.float8e4
ALU = mybir.AluOpType
ACTF = mybir.ActivationFunctionType
DR = mybir.MatmulPerfMode.DoubleRow

QK_SCALE = 64.0      # host-side scale on Wq/Wk (+ biases) before fp8 cast

_CACHED_NC = None


def build_nc():
    nc = bacc.Bacc(None, target_bir_lowering=False)

    # bf16 x tiles for the V projection; fp8 x tiles (with ones/zero bias
    # chunks 8/9) for the Q/K DoubleRow projections.
    xT_d = nc.dram_tensor("xt", [NT, P, KC, P], BF16, kind="ExternalInput")
    x8_d = nc.dram_tensor("x8", [NT, P, KC + 2, P], FP8, kind="ExternalInput")
    wq_d = nc.dram_tensor("wq", [P, KC + 2, D], FP8, kind="ExternalInput")
    wk_d = nc.dram_tensor("wk", [P, KC + 2, D], FP8, kind="ExternalInput")
    wv_d = nc.dram_tensor("wv", [P, KC + 1, D], BF16, kind="ExternalInput")
    wo_d = nc.dram_tensor("wo", [P, KC + 1, D], BF16, kind="ExternalInput")
    onesbf_d = nc.dram_tensor("onesbf", [P, P], BF16, kind="ExternalInput")
    out_d = nc.dram_tensor("out", [N, D], F32, kind="ExternalOutput")

    with tile.TileContext(nc) as tc:
        with (
            tc.tile_pool(name="consts", bufs=1) as consts,
            tc.tile_pool(name="xt", bufs=3) as xt_pool,
            tc.tile_pool(name="x8", bufs=3) as x8_pool,
            tc.tile_pool(name="qk", bufs=3) as qk_pool,
            tc.tile_pool(name="v", bufs=3) as v_pool,
            tc.tile_pool(name="att", bufs=3) as att_pool,
            tc.tile_pool(name="attT", bufs=3) as attT_pool,
            tc.tile_pool(name="o", bufs=2) as o_pool,
            tc.tile_pool(name="small", bufs=3) as small,
            tc.tile_pool(name="psum", bufs=2, space="PSUM") as psum,
        ):
            wq_sb = consts.tile([P, KC + 2, D], FP8)
            wk_sb = consts.tile([P, KC + 2, D], FP8)
            wv_sb = consts.tile([P, KC + 1, D], BF16)
            wo_sb = consts.tile([P, KC + 1, D], BF16)

            ident = consts.tile([P, P], BF16)
            make_identity(nc, ident)

            # lhsT for the V/O bias matmuls: partition 0 all-ones, rest zero,
            # so out[m, n] += rhs[0, n] broadcasts the bias row over tokens.
            ones_bf = consts.tile([P, P], BF16)
            nc.sync.dma_start(ones_bf, onesbf_d[:])

            # Weight DMAs split into column halves, Q/K (critical path for
            # tile 0) first, so the first projections start ~4us in instead
            # of waiting ~25us for all weights to land.
            for half in range(2):
                sl = slice(half * 512, (half + 1) * 512)
                nc.sync.dma_start(wq_sb[:, :, sl], wq_d[:, :, sl])
                nc.sync.dma_start(wk_sb[:, :, sl], wk_d[:, :, sl])
                nc.sync.dma_start(wv_sb[:, :, sl], wv_d[:, :, sl])
            for half in range(2):
                sl = slice(half * 512, (half + 1) * 512)
                nc.sync.dma_start(wo_sb[:, :, sl], wo_d[:, :, sl])

            # Software-pipelined: tile t's transpose + O-projection are
            # emitted one iteration late, so the in-order PE queue runs
            # QKV(t+1) while tile t's attention chain (DVE/Act) completes.
            atts = {}

            def emit_tail(t):
                att = atts.pop(t)
                attT = attT_pool.tile([P, KC, P], BF16)
                pst = psum.tile([P, KC, P], BF16, tag="ps_tr")
                for k in range(KC):
                    nc.tensor.transpose(
                        pst[:, k, :],
                        att[:, k * P:(k + 1) * P],
                        ident,
                    )
                nc.scalar.copy(attT, pst)

                # ---- O-projection (bf16) with fused bias row ----
                o_sb = o_pool.tile([P, D], F32)
                for half in range(2):
                    sl = slice(half * 512, (half + 1) * 512)
                    ps = psum.tile([P, 512], F32, tag="ps_o")
                    for k in range(KC):
                        nc.tensor.matmul(
                            ps, attT[:, k, :], wo_sb[:, k, sl],
                            start=(k == 0), stop=False,
                        )
                    nc.tensor.matmul(
                        ps, ones_bf, wo_sb[:, KC, sl], start=False, stop=True,
                    )
                    nc.scalar.copy(o_sb[:, sl], ps)

                nc.sync.dma_start(out_d[t * P:(t + 1) * P, :], o_sb)

            for t in range(NT):
                xt = xt_pool.tile([P, KC, P], BF16)
                nc.sync.dma_start(xt, xT_d[t])
                x8 = x8_pool.tile([P, KC + 2, P], FP8)
                nc.sync.dma_start(x8, x8_d[t])

                # ---- Q/K projections: fp8 DoubleRow, bias in the 5th MM ----
                q_sb = qk_pool.tile([P, D], BF16, tag="q")
                k_sb = qk_pool.tile([P, D], BF16, tag="k")
                for w_sb, dst in ((wq_sb, q_sb), (wk_sb, k_sb)):
                    for half in range(2):
                        sl = slice(half * 512, (half + 1) * 512)
                        ps = psum.tile([P, 512], F32, tag="ps_qkv", bufs=3)
                        for k in range(0, KC + 2, 2):
                            nc.tensor.matmul(
                                ps, x8[:, k:k + 2, :], w_sb[:, k:k + 2, sl],
                                start=(k == 0), stop=(k == KC),
                                perf_mode=DR,
                            )
                        nc.scalar.copy(dst[:, sl], ps)

                # ---- V projection (bf16) with fused bias row ----
                v_sb = v_pool.tile([P, D], BF16, tag="v")
                for half in range(2):
                    sl = slice(half * 512, (half + 1) * 512)
                    ps = psum.tile([P, 512], F32, tag="ps_qkv", bufs=3)
                    for k in range(KC):
                        nc.tensor.matmul(
                            ps, xt[:, k, :], wv_sb[:, k, sl],
                            start=(k == 0), stop=False,
                        )
                    nc.tensor.matmul(
                        ps, ones_bf, wv_sb[:, KC, sl], start=False, stop=True,
                    )
                    nc.scalar.copy(v_sb[:, sl], ps)

                # ---- per-token 4x4 head-head scores (DVE) ----
                s_sb = small.tile([P, 16], F32, tag="s")
                for h in range(NUM_HEADS):
                    for g in range(NUM_HEADS):
                        prod = small.tile([P, HEAD_DIM], BF16, tag="prod")
                        nc.vector.scalar_tensor_tensor(
                            out=prod,
                            in0=q_sb[:, h * HEAD_DIM:(h + 1) * HEAD_DIM],
                            scalar=1.0 / (math.sqrt(HEAD_DIM) * QK_SCALE * QK_SCALE),
                            in1=k_sb[:, g * HEAD_DIM:(g + 1) * HEAD_DIM],
                            op0=ALU.mult,
                            op1=ALU.mult,
                            accum_out=s_sb[:, h * 4 + g: h * 4 + g + 1],
                        )

                # ---- softmax over g (scores are O(1); no max-subtract) ----
                e_sb = small.tile([P, 16], F32, tag="e")
                nc.scalar.activation(e_sb, s_sb, ACTF.Exp)
                sums = small.tile([P, NUM_HEADS], F32, tag="sums")
                nc.vector.tensor_reduce(
                    out=sums,
                    in_=e_sb.rearrange("p (h g) -> p h g", g=NUM_HEADS),
                    axis=mybir.AxisListType.X,
                    op=ALU.add,
                )
                rec = small.tile([P, NUM_HEADS], F32, tag="rec")
                nc.vector.reciprocal(rec, sums)
                w_sb = small.tile([P, 16], F32, tag="w")
                nc.vector.tensor_tensor(
                    out=w_sb.rearrange("p (h g) -> p h g", g=NUM_HEADS),
                    in0=e_sb.rearrange("p (h g) -> p h g", g=NUM_HEADS),
                    in1=rec[:, :, None].to_broadcast((P, NUM_HEADS, NUM_HEADS)),
                    op=ALU.mult,
                )

                # ---- head mixing: att_h = sum_g w[h,g] * V_g (DVE) ----
                att = att_pool.tile([P, D], BF16)
                for h in range(NUM_HEADS):
                    hs = slice(h * HEAD_DIM, (h + 1) * HEAD_DIM)
                    nc.vector.tensor_scalar_mul(
                        att[:, hs], v_sb[:, 0:HEAD_DIM], w_sb[:, 4 * h: 4 * h + 1],
                    )
                    for g in range(1, NUM_HEADS):
                        nc.vector.scalar_tensor_tensor(
                            out=att[:, hs],
                            in0=v_sb[:, g * HEAD_DIM:(g + 1) * HEAD_DIM],
                            scalar=w_sb[:, 4 * h + g: 4 * h + g + 1],
                            in1=att[:, hs],
                            op0=ALU.mult,
                            op1=ALU.add,
                        )
                atts[t] = att

                if t > 0:
                    emit_tail(t - 1)
            emit_tail(NT - 1)

    nc.compile()
    return nc


def _prep_inputs(x, Wq, bq, Wk, bk, Wv, bv, Wo, bo):
    """Per-core input maps: x tiles (bf16 + fp8) + replicated weights."""
    x = np.asarray(x, dtype=np.float32)
    FP8NP = ml_dtypes.float8_e4m3

    def aug_bf(W, b):
        return np.ascontiguousarray(np.concatenate(
            [np.asarray(W, np.float32),
             np.asarray(b, np.float32)[None, :],
             np.zeros((P - 1, D), np.float32)], axis=0,
        ).reshape(KC + 1, P, D).transpose(1, 0, 2)).astype(ml_dtypes.bfloat16)

    def aug_fp8(W, b):
        # chunks 0..7: 64*W ; chunk 8: 64*bias row ; chunk 9: zeros
        return np.ascontiguousarray(np.concatenate(
            [np.asarray(W, np.float32) * QK_SCALE,
             np.asarray(b, np.float32)[None, :] * QK_SCALE,
             np.zeros((2 * P - 1, D), np.float32)], axis=0,
        ).reshape(KC + 2, P, D).transpose(1, 0, 2)).astype(FP8NP)

    wq_h = aug_fp8(Wq, bq)
    wk_h = aug_fp8(Wk, bk)
    wv_h = aug_bf(Wv, bv)
    wo_h = aug_bf(Wo, bo)

    onesbf_h = np.zeros((P, P), np.float32)
    onesbf_h[0, :] = 1.0
    onesbf_h = onesbf_h.astype(ml_dtypes.bfloat16)

    in_maps = []
    for b in range(B):
        xt32 = np.ascontiguousarray(
            x[b].T.reshape(KC, P, NT, P).transpose(2, 1, 0, 3))  # [NT,P,KC,P]
        xt = xt32.astype(ml_dtypes.bfloat16)
        # fp8 copy with two extra chunks: 8 = partition-0 ones, 9 = zeros
        x8 = np.zeros((NT, P, KC + 2, P), np.float32)
        x8[:, :, :KC, :] = xt32
        x8[:, 0, KC, :] = 1.0
        x8 = x8.astype(FP8NP)
        in_maps.append({
            "xt": xt, "x8": x8, "wq": wq_h, "wk": wk_h, "wv": wv_h,
            "wo": wo_h, "onesbf": onesbf_h,
        })
    return in_maps


def kernel(**inputs):
    global _CACHED_NC
    if _CACHED_NC is None:
        _CACHED_NC = build_nc()
    nc = _CACHED_NC

    in_maps = _prep_inputs(
        inputs["x"],
        inputs["Wq"], inputs["bq"],
        inputs["Wk"], inputs["bk"],
        inputs["Wv"], inputs["bv"],
        inputs["Wo"], inputs["bo"],
    )
    res = run_bass_kernel_spmd(nc, in_maps, core_ids=list(range(B)))
    out = np.stack([r["out"] for r in res.results], axis=0)
    return out.astype(np.float32)
